# revision 13
# baseline (speedup 1.0000x reference)
"""Multi-head attention (N=4, L=2048, E=1024, H=16) on 8 Trainium2 cores.

Sharding: core c -> (batch n = c // 2, head-group g = c % 2).  Each core
computes, for its batch and its 8 heads (512 embed dims):
  qp_T/kp_T = (W x^T) in [d, tok] layout (fp8 DoubleRow matmuls; the x64
  fp8 weight scale folds into the softmax exp scale), vp in [tok, d]
  layout (fp16), S_T[k, q] scores with two heads packed in the 128
  partitions via PE row tiling, exp via ACT, attn@v with a ones column
  appended to vp so the softmax denominator accumulates in the same PSUM
  tile, normalization via DVE reciprocal_approx_fast + a col-tiled PE
  replicate matmul + one DVE multiply, then the output projection (fp16).
Host sums the two per-group partial outputs per batch and adds bo.

All DRAM inputs are pre-swizzled on the host into the device layout
([partition, ...] contiguous) so every DMA moves >=4KB contiguous runs
per partition instead of 512B strided descriptors.

Pipeline structure: the softmax exp stream on the ACT engine is the
critical path (~256 x ~1.05us).  All other work (v projection, next
pair's q/k projections, the output projection, normalization) is
interleaved into the attention blocks as PE/DVE filler so ACT never
waits.
"""

import os

import numpy as np

import concourse.bacc as bacc
import concourse.mybir as mybir
import concourse.tile as tile
from concourse.bass import ds, ts
from concourse.bass_utils import run_bass_kernel_spmd

F32 = mybir.dt.float32
F16 = mybir.dt.float16
F8 = mybir.dt.float8e4

E = 1024          # embed
H = 16            # heads (global)
D = 64            # head dim
L = 2048          # sequence length
NB = 4            # batch
GE = 512          # embed dims per head group (8 heads)
P = 128           # partitions
TB = L // 512     # 4 token blocks of 512
EC = E // P       # 8 embed chunks
DC = GE // P      # 4 d-chunks per group == head pairs
KT = L // P       # 16 key-token chunks
LAG = 2           # attnv trails scores by LAG kt-chunks
WS = 64.0         # fp8 weight scale for q/k projections

_CACHE = {}


def _build():
    nc = bacc.Bacc("TRN2", debug=False, enable_asserts=False, num_devices=8)

    # device-layout tensors (host pre-swizzles)
    xq = nc.dram_tensor("xq", [P, TB, EC, 512], F8, kind="ExternalInput").ap()
    xk = nc.dram_tensor("xk", [P, TB, EC, 512], F8, kind="ExternalInput").ap()
    xv = nc.dram_tensor("xv", [P, TB, EC, 512], F16, kind="ExternalInput").ap()
    wq = nc.dram_tensor("wq", [P, EC, GE], F8, kind="ExternalInput").ap()
    wk = nc.dram_tensor("wk", [P, EC, GE], F8, kind="ExternalInput").ap()
    wv = nc.dram_tensor("wv", [P, EC, GE], F16, kind="ExternalInput").ap()
    wo = nc.dram_tensor("wo", [P, DC, E], F16, kind="ExternalInput").ap()
    bqk = nc.dram_tensor("bqk", [2, P, DC], F32, kind="ExternalInput").ap()
    bvr = nc.dram_tensor("bvr", [1, GE], F16, kind="ExternalInput").ap()
    out = nc.dram_tensor("out", [L, E], F32, kind="ExternalOutput").ap()

    with tile.TileContext(nc) as tc, \
         nc.allow_low_precision(reason="fp16/fp8 attention internals by design"):
        with tc.tile_pool(name="persist", bufs=1) as pp, \
             tc.tile_pool(name="wpool", bufs=1) as wp, \
             tc.tile_pool(name="xpool", bufs=8) as xp, \
             tc.tile_pool(name="vxpool", bufs=4) as vxp, \
             tc.tile_pool(name="bias", bufs=1) as bp, \
             tc.tile_pool(name="expp", bufs=6) as ep, \
             tc.tile_pool(name="nrm", bufs=2) as npool, \
             tc.tile_pool(name="otmp", bufs=3) as ot, \
             tc.tile_pool(name="ppsum", bufs=2, space="PSUM") as pps, \
             tc.tile_pool(name="spsum", bufs=2, space="PSUM") as sps, \
             tc.tile_pool(name="opsum", bufs=1, space="PSUM") as ops:
            # persistent SBUF
            vp = pp.tile([P, KT, 8, D + 1], F16)         # vp_aug per head
            ao = pp.tile([P, DC, L], F16)                # normalized attnout_T
            qs = pp.tile([P, DC, L], F16)                # qp_T  [d, pair, tok]
            ks = pp.tile([P, DC, L], F16)                # kp_T
            ones32 = pp.tile([1, P], F32)
            ones = pp.tile([1, P], F16)
            nc.gpsimd.memset(ones32[:], 1.0)
            nc.vector.tensor_copy(ones[:], ones32[:])

            bq_t = bp.tile([P, DC], F32, tag="bq")
            bk_t = bp.tile([P, DC], F32, tag="bk")
            bv_row = bp.tile([1, GE], F16, tag="bv")
            nc.sync.dma_start(bq_t[:], bqk[0])
            nc.sync.dma_start(bk_t[:], bqk[1])
            nc.sync.dma_start(bv_row[:], bvr)

            wq_sb = wp.tile([P, EC, GE], F8, tag="wq")
            wk_sb = wp.tile([P, EC, GE], F8, tag="wk")
            wv_sb = wp.tile([P, EC, GE], F16, tag="wv")
            wo_sb = wp.tile([P, DC, E], F16, tag="wo")
            # stripe bulk DMAs over several queues (one dma_start = one
            # queue at ~22 GB/s; striping restores wire-rate loading)
            for j in range(4):
                nc.sync.dma_start(
                    wk_sb[:, 2 * j : 2 * j + 2, :], wk[:, 2 * j : 2 * j + 2, :]
                )
            for j in range(4):
                nc.sync.dma_start(
                    wq_sb[:, 2 * j : 2 * j + 2, :], wq[:, 2 * j : 2 * j + 2, :]
                )
            for j in range(4):
                nc.sync.dma_start(
                    wv_sb[:, 2 * j : 2 * j + 2, :], wv[:, 2 * j : 2 * j + 2, :]
                )
            for j in range(4):
                nc.sync.dma_start(
                    wo_sb[:, j : j + 1, :], wo[:, j : j + 1, :]
                )

            # ones column of vp_aug (softmax denominator accumulator)
            onescol = bp.tile([P, KT], F32, tag="onescol")
            nc.gpsimd.memset(onescol[:], 1.0)
            nc.vector.tensor_copy(
                vp[:, :, :, D : D + 1],
                onescol[:, :, None, None].to_broadcast([P, KT, 8, 1]),
            )

            # ---------------- emission helpers ----------------
            def load_slab(x_ap, tb):
                x_sb = xp.tile([P, EC, 512], F8, tag="xslab", name="x_sb")
                for j in range(4):
                    nc.sync.dma_start(
                        x_sb[:, 2 * j : 2 * j + 2, :],
                        x_ap[:, tb, 2 * j : 2 * j + 2, :],
                    )
                return x_sb

            def load_vslab(tb):
                x_sb = vxp.tile([P, EC, 512], F16, tag="vslab", name="v_sb")
                for j in range(4):
                    nc.sync.dma_start(
                        x_sb[:, 2 * j : 2 * j + 2, :],
                        xv[:, tb, 2 * j : 2 * j + 2, :],
                    )
                return x_sb

            def qk_chunk(pr, st, w_sb, b_t, tb, slab):
                # fp8 DoubleRow: 2 contraction rows per PE cell
                ps_t = pps.tile([P, 512], F32, tag="pp")
                for e2 in range(EC // 2):
                    nc.tensor.matmul(
                        ps_t[:],
                        w_sb[:, 2 * e2 : 2 * e2 + 2, ts(pr, P)],
                        slab[:, 2 * e2 : 2 * e2 + 2, :],
                        start=(e2 == 0),
                        stop=(e2 == EC // 2 - 1),
                        perf_mode=mybir.MatmulPerfMode.DoubleRow,
                    )
                nc.vector.tensor_scalar_add(
                    st[:, pr, ts(tb, 512)], ps_t[:], b_t[:, pr : pr + 1]
                )

            def vproj_chunk(c, slab):
                ps_t = pps.tile([P, 512], F32, tag="pp")
                for e in range(EC):
                    nc.tensor.matmul(
                        ps_t[:],
                        slab[:, e, ts(c % 4, P)],
                        wv_sb[:, e, :],
                        start=(e == 0),
                        stop=False,
                    )
                nc.tensor.matmul(
                    ps_t[:], ones[:, :P], bv_row[:], start=False, stop=True
                )
                nc.vector.tensor_copy(
                    vp[:, c, :, 0:D],
                    ps_t.rearrange("p (h d) -> p h d", d=D),
                )

            def oproj_piece(tb, ob):
                ps_f = pps.tile([P, 512], F32, tag="pp")
                for dc in range(DC):
                    nc.tensor.matmul(
                        ps_f[:],
                        ao[:, dc, ts(tb, P)],
                        wo_sb[:, dc, ts(ob, 512)],
                        start=(dc == 0),
                        stop=(dc == DC - 1),
                    )
                o_t = ot.tile([P, 512], F32, tag="fout")
                nc.vector.tensor_copy(o_t[:], ps_f[:])
                for j in range(2):
                    nc.sync.dma_start(
                        out[ts(tb, P), ds(ob * 512 + 256 * j, 256)],
                        o_t[:, ds(256 * j, 256)],
                    )

            # normalization part 1 (block end): drain attnv PSUM into SBUF
            def norm_gather(ps_oo):
                sbp = npool.tile([P, 512], F32, tag="sbp", name="sbp")
                dn = npool.tile([1, 2, 512], F32, tag="dn", name="dn")
                nc.vector.tensor_copy(sbp[0:D, :], ps_oo[0][0:D, :])
                nc.vector.tensor_copy(sbp[D:P, :], ps_oo[1][0:D, :])
                nc.vector.tensor_copy(dn[:, 0, :], ps_oo[0][D : D + 1, :])
                nc.vector.tensor_copy(dn[:, 1, :], ps_oo[1][D : D + 1, :])
                return sbp, dn

            # normalization part 2 (mid next block): reciprocal + replicate
            def norm_finish(pr, qb, sbp, dn):
                dninv = npool.tile([1, 2, 512], F32, tag="dninv", name="dninv")
                nc.vector.reciprocal_approx_fast(dninv[:], dn[:])
                dinv = npool.tile([1, 2, 512], F16, tag="dinv", name="dinv")
                nc.vector.tensor_copy(dinv[:], dninv[:])
                ps_r = pps.tile([P, 512], F32, tag="pp", name="ps_r")
                nc.tensor.matmul(
                    ps_r[0:D, :], ones[:, :D], dinv[:, 0, :],
                    start=True, stop=True,
                )
                nc.tensor.matmul(
                    ps_r[D:P, :], ones[:, :D], dinv[:, 1, :],
                    start=True, stop=True, tile_position=(0, 64),
                )
                nc.vector.tensor_tensor(
                    ao[:, pr, ts(qb, 512)],
                    sbp[:],
                    ps_r[:],
                    mybir.AluOpType.mult,
                )

            # ---------------- startup ----------------
            # DMA priority order: the minimal set gating the first scores
            # (wk + k-slab 0 + wq + q-slab 0) goes first, then the tensors
            # needed progressively later.  Everything else (kproj tb1-3,
            # remaining qproj, the whole v projection) runs inside block
            # (0,0) as PE filler while the exp stream ramps.
            kslabs = [load_slab(xk, 0)]
            qslab0 = load_slab(xq, 0)
            vslabs = {0: load_vslab(0)}
            kslabs += [load_slab(xk, tb) for tb in range(1, TB)]
            vslabs[1] = load_vslab(1)
            qslabs_rest = {tb: load_slab(xq, tb) for tb in range(1, TB)}
            vslabs[2] = load_vslab(2)
            vslabs[3] = load_vslab(3)
            qk_chunk(0, ks, wk_sb, bk_t, 0, kslabs[0])
            qk_chunk(0, qs, wq_sb, bq_t, 0, qslab0)

            # block (0,0) filler sequence: v-projection chunks, remaining
            # k-projection (tb t needed by scores kt>=4t) and q-projection
            # (needed by blocks (0,1..3)); emitted 2 per kt
            blk00_tasks = []
            for c in range(16):
                blk00_tasks.append(
                    lambda c=c: vproj_chunk(c, vslabs[c // 4])
                )
                if c in (1, 3, 5):
                    tb = (c + 1) // 2
                    blk00_tasks.append(
                        lambda tb=tb: qk_chunk(
                            0, ks, wk_sb, bk_t, tb, kslabs[tb]
                        )
                    )
                if c in (7, 9, 11):
                    tb = (c - 5) // 2
                    blk00_tasks.append(
                        lambda tb=tb: qk_chunk(
                            0, qs, wq_sb, bq_t, tb, qslabs_rest[tb]
                        )
                    )

            # ---------------- attention with interleaved fillers ----------
            pending_norm = None     # (pr, qb, sbp, dn)

            for pr in range(DC):
                # filler tasks for this pair's 4 blocks: list of callables
                fillers = [[] for _ in range(TB)]
                if pr < DC - 1:
                    # next pair's q/k projection: 8 chunks
                    npr = pr + 1
                    chunks = []
                    for x_ap, w_sb, b_t, st in [
                        (xk, wk_sb, bk_t, ks),
                        (xq, wq_sb, bq_t, qs),
                    ]:
                        for tb in range(TB):
                            chunks.append((x_ap, w_sb, b_t, st, tb))

                    def mk_qk(npr, x_ap, w_sb, b_t, st, tb):
                        holder = {}

                        def prefetch():
                            holder["slab"] = load_slab(x_ap, tb)

                        def run():
                            qk_chunk(npr, st, w_sb, b_t, tb, holder["slab"])

                        return prefetch, run

                    start_qb = 0 if pr > 0 else 1
                    tasks = [(mk_qk(npr, *ch)) for ch in chunks]
                    nblk = TB - start_qb
                    for i, (pf, run) in enumerate(tasks):
                        blk = start_qb + min(i * nblk // len(tasks), nblk - 1)
                        fillers[blk].append((pf, run))
                else:
                    # pair 3: output projection for finished q-blocks
                    def mk_op(tb, ob):
                        return (None, lambda: oproj_piece(tb, ob))

                    for j in range(TB - 1):      # oproj for qb j in block j+1
                        for tb in range(4 * j, 4 * j + 4):
                            for ob in range(2):
                                fillers[j + 1].append(mk_op(tb, ob))

                for qb in range(TB):
                    # emit prefetches for this block's fillers up front
                    for pf, _ in fillers[qb]:
                        if pf is not None:
                            pf()
                    fq = [run for _, run in fillers[qb]]
                    fi = 0

                    ps_oo = [
                        ops.tile([P, 512], F32, tag=f"ov{i}", name=f"ov{i}")
                        for i in range(2)
                    ]
                    ets = {}
                    first_blk = pr == 0 and qb == 0
                    lag = 4 if first_blk else LAG
                    for kt in range(KT):
                        # scores for both heads of the pair (row-tiled pair)
                        ps_s = sps.tile([P, 1024], F32, tag="sc")
                        for i in range(2):
                            nc.tensor.matmul(
                                ps_s[:, ts(i, 512)],
                                ks[ds(64 * i, 64), pr, ts(kt, P)],
                                qs[ds(64 * i, 64), pr, ts(qb, 512)],
                                start=True,
                                stop=True,
                                tile_position=(64 * i, 0),
                            )
                        e_t = ep.tile([P, 1024], F16, tag="exp", name="e_t")
                        nc.scalar.activation(
                            e_t[:],
                            ps_s[:],
                            mybir.ActivationFunctionType.Exp,
                            scale=float(1.0 / (32.0 * WS * WS)),
                        )
                        ets[kt] = e_t

                        if first_blk:
                            # v projection + remaining k/q projection race
                            # ahead of the attnv consumption (lag 4)
                            for _ in range(2):
                                if blk00_tasks:
                                    blk00_tasks.pop(0)()
                        # pair 3's oproj fillers read ao written by
                        # norm_finish, so it must be emitted before them
                        nf_kt = 1 if pr == DC - 1 else 5
                        if kt == nf_kt and pending_norm is not None:
                            norm_finish(*pending_norm)
                            pending_norm = None
                        if (
                            kt % 2 == 1
                            and (pr != DC - 1 or kt >= 3)
                            and fi < len(fq)
                        ):
                            fq[fi]()
                            fi += 1
                        if kt >= lag:
                            k2 = kt - lag
                            for i in range(2):
                                nc.tensor.matmul(
                                    ps_oo[i][0 : D + 1, :],
                                    vp[:, k2, 2 * pr + i, :],
                                    ets[k2][:, ts(i, 512)],
                                    start=(k2 == 0),
                                    stop=False,
                                )
                    if first_blk:
                        while blk00_tasks:
                            blk00_tasks.pop(0)()
                    for k2 in range(KT - lag, KT):
                        for i in range(2):
                            nc.tensor.matmul(
                                ps_oo[i][0 : D + 1, :],
                                vp[:, k2, 2 * pr + i, :],
                                ets[k2][:, ts(i, 512)],
                                start=False,
                                stop=(k2 == KT - 1),
                            )
                    while fi < len(fq):
                        fq[fi]()
                        fi += 1
                    sbp, dn = norm_gather(ps_oo)
                    pending_norm = (pr, qb, sbp, dn)

            # tail: last block's normalization + its output projection
            norm_finish(*pending_norm)
            pending_norm = None
            for tb in range(4 * (TB - 1), 4 * TB):
                for ob in range(2):
                    oproj_piece(tb, ob)

    nc.compile()
    return nc


def _swizzle_x(xt, dtype):
    # [E, L] -> [P, TB, EC, 512]  with E = eo*P + p, L = tb*512 + t
    arr = xt.reshape(EC, P, TB, 512).transpose(1, 2, 0, 3)
    return np.ascontiguousarray(arr.astype(dtype))


def _swizzle_w(wt, dtype, inner):
    # [E_in, F] -> [P, E_in//P, F]
    arr = wt.reshape(inner, P, wt.shape[1]).transpose(1, 0, 2)
    return np.ascontiguousarray(arr.astype(dtype))


def kernel(q, k, v, padding_mask, sequence_mask, Wq, bq, Wk, bk, Wv, bv, Wo, bo):
    # masks intentionally unused: the reference discards masked_fill results.
    import ml_dtypes

    F8NP = ml_dtypes.float8_e4m3

    if "nc" not in _CACHE:
        _CACHE["nc"] = _build()
    nc = _CACHE["nc"]

    q = np.asarray(q, np.float32)
    k = np.asarray(k, np.float32)
    v = np.asarray(v, np.float32)
    Wq = np.asarray(Wq, np.float32)
    Wk = np.asarray(Wk, np.float32)
    Wv = np.asarray(Wv, np.float32)
    Wo = np.asarray(Wo, np.float32)
    bq = np.asarray(bq, np.float32)
    bk = np.asarray(bk, np.float32)
    bv = np.asarray(bv, np.float32)
    bo = np.asarray(bo, np.float32)

    in_maps = []
    for c in range(8):
        n, g = c // 2, c % 2
        sl = slice(g * GE, (g + 1) * GE)
        # q/k projections run in fp8 with weights pre-scaled by WS; the
        # scale cancels inside the softmax (folded into the exp scale).
        bqk_arr = np.stack(
            [
                (WS * bq[sl]).reshape(DC, P).T,
                (WS * bk[sl]).reshape(DC, P).T,
            ]
        ).astype(np.float32)
        in_maps.append(
            {
                "xq": _swizzle_x(q[n].T, F8NP),
                "xk": _swizzle_x(k[n].T, F8NP),
                "xv": _swizzle_x(v[n].T, np.float16),
                "wq": _swizzle_w(WS * Wq[sl, :].T, F8NP, EC),
                "wk": _swizzle_w(WS * Wk[sl, :].T, F8NP, EC),
                "wv": _swizzle_w(Wv[sl, :].T, np.float16, EC),
                "wo": _swizzle_w(Wo[:, sl].T, np.float16, DC),
                "bqk": np.ascontiguousarray(bqk_arr),
                "bvr": np.ascontiguousarray(bv[sl][None, :].astype(np.float16)),
            }
        )

    trace = os.environ.get("KERNEL_TRACE") == "1"
    kw = {}
    if trace:
        kw = dict(trace=True, trace_cores=list(range(8)))
    res = run_bass_kernel_spmd(nc, in_maps, core_ids=list(range(8)), **kw)
    if trace:
        _CACHE["exec_time_ns"] = res.exec_time_ns
        _CACHE["mean_exec_time_ns"] = res.mean_exec_time_ns

    outp = np.empty((NB, L, E), np.float32)
    for n in range(NB):
        outp[n] = (
            res.results[2 * n]["out"] + res.results[2 * n + 1]["out"] + bo[None, :]
        )
    return outp


# revision 17
# speedup vs baseline: 1.0197x; 1.0197x over previous
"""Multi-head attention (N=4, L=2048, E=1024, H=16) on 8 Trainium2 cores.

Sharding: core c -> (batch n = c // 2, head-group g = c % 2).  Each core
computes, for its batch and its 8 heads (512 embed dims):
  qp_T/kp_T = (W x^T) in [d, tok] layout (fp8 DoubleRow matmuls; the x64
  fp8 weight scale folds into the softmax exp scale), vp in [tok, d]
  layout (fp16), S_T[k, q] scores with two heads packed in the 128
  partitions via PE row tiling, exp via ACT, attn@v with a ones column
  appended to vp so the softmax denominator accumulates in the same PSUM
  tile, normalization via DVE reciprocal_approx_fast + a col-tiled PE
  replicate matmul + one DVE multiply, then the output projection (fp16).
Host sums the two per-group partial outputs per batch and adds bo.

All DRAM inputs are pre-swizzled on the host into the device layout
([partition, ...] contiguous) so every DMA moves >=4KB contiguous runs
per partition instead of 512B strided descriptors.

Pipeline structure: the softmax exp stream on the ACT engine is the
critical path (~256 x ~1.05us).  All other work (v projection, next
pair's q/k projections, the output projection, normalization) is
interleaved into the attention blocks as PE/DVE filler so ACT never
waits.
"""

import os

import numpy as np

import concourse.bacc as bacc
import concourse.mybir as mybir
import concourse.tile as tile
from concourse.bass import ds, ts
from concourse.bass_utils import run_bass_kernel_spmd

F32 = mybir.dt.float32
F16 = mybir.dt.float16
F8 = mybir.dt.float8e4

E = 1024          # embed
H = 16            # heads (global)
D = 64            # head dim
L = 2048          # sequence length
NB = 4            # batch
GE = 512          # embed dims per head group (8 heads)
P = 128           # partitions
TB = L // 512     # 4 token blocks of 512
EC = E // P       # 8 embed chunks
DC = GE // P      # 4 d-chunks per group == head pairs
KT = L // P       # 16 key-token chunks
LAG = 2           # attnv trails scores by LAG kt-chunks
WS = 64.0         # fp8 weight scale for q/k projections

_CACHE = {}


def _build():
    nc = bacc.Bacc("TRN2", debug=False, enable_asserts=False, num_devices=8)

    # device-layout tensors (host pre-swizzles)
    xq = nc.dram_tensor("xq", [P, TB, EC, 512], F8, kind="ExternalInput").ap()
    xk = nc.dram_tensor("xk", [P, TB, EC, 512], F8, kind="ExternalInput").ap()
    xv = nc.dram_tensor("xv", [P, TB, EC, 512], F16, kind="ExternalInput").ap()
    wq = nc.dram_tensor("wq", [P, EC, GE], F8, kind="ExternalInput").ap()
    wk = nc.dram_tensor("wk", [P, EC, GE], F8, kind="ExternalInput").ap()
    wv = nc.dram_tensor("wv", [P, EC, GE], F16, kind="ExternalInput").ap()
    wo = nc.dram_tensor("wo", [P, DC, E], F16, kind="ExternalInput").ap()
    bqk = nc.dram_tensor("bqk", [2, P, DC], F32, kind="ExternalInput").ap()
    bvr = nc.dram_tensor("bvr", [1, GE], F16, kind="ExternalInput").ap()
    out = nc.dram_tensor("out", [L, E], F32, kind="ExternalOutput").ap()

    with tile.TileContext(nc) as tc, \
         nc.allow_low_precision(reason="fp16/fp8 attention internals by design"):
        with tc.tile_pool(name="persist", bufs=1) as pp, \
             tc.tile_pool(name="wpool", bufs=1) as wp, \
             tc.tile_pool(name="xpool", bufs=8) as xp, \
             tc.tile_pool(name="vxpool", bufs=4) as vxp, \
             tc.tile_pool(name="bias", bufs=1) as bp, \
             tc.tile_pool(name="expp", bufs=6) as ep, \
             tc.tile_pool(name="nrm", bufs=2) as npool, \
             tc.tile_pool(name="otmp", bufs=3) as ot, \
             tc.tile_pool(name="ppsum", bufs=2, space="PSUM") as pps, \
             tc.tile_pool(name="spsum", bufs=2, space="PSUM") as sps, \
             tc.tile_pool(name="opsum", bufs=1, space="PSUM") as ops:
            # persistent SBUF
            vp = pp.tile([P, KT, 8, D + 1], F16)         # vp_aug per head
            ao = pp.tile([P, DC, L], F16)                # normalized attnout_T
            qs = pp.tile([P, DC, L], F16)                # qp_T  [d, pair, tok]
            ks = pp.tile([P, DC, L], F16)                # kp_T
            ones32 = pp.tile([1, P], F32)
            ones = pp.tile([1, P], F16)
            nc.gpsimd.memset(ones32[:], 1.0)
            nc.vector.tensor_copy(ones[:], ones32[:])

            bq_t = bp.tile([P, DC], F32, tag="bq")
            bk_t = bp.tile([P, DC], F32, tag="bk")
            bv_row = bp.tile([1, GE], F16, tag="bv")

            wq_sb = wp.tile([P, EC, GE], F8, tag="wq")
            wk_sb = wp.tile([P, EC, GE], F8, tag="wk")
            wv_sb = wp.tile([P, EC, GE], F16, tag="wv")
            wo_sb = wp.tile([P, DC, E], F16, tag="wo")
            # weight/bias DMAs are emitted in the startup section below in
            # criticality order: each dma_start costs ~0.6us of *serial*
            # issue time on the Sync engine, so the loads gating the first
            # scores must be first in the issue stream.

            # ones column of vp_aug (softmax denominator accumulator)
            onescol = bp.tile([P, KT], F32, tag="onescol")
            nc.gpsimd.memset(onescol[:], 1.0)
            nc.vector.tensor_copy(
                vp[:, :, :, D : D + 1],
                onescol[:, :, None, None].to_broadcast([P, KT, 8, 1]),
            )

            # ---------------- emission helpers ----------------
            def load_slab(x_ap, tb):
                x_sb = xp.tile([P, EC, 512], F8, tag="xslab", name="x_sb")
                nc.sync.dma_start(x_sb[:], x_ap[:, tb])
                return x_sb

            def load_vslab(tb):
                x_sb = vxp.tile([P, EC, 512], F16, tag="vslab", name="v_sb")
                nc.sync.dma_start(x_sb[:], xv[:, tb])
                return x_sb

            def qk_chunk(pr, st, w_sb, b_t, tb, slab):
                # fp8 DoubleRow: 2 contraction rows per PE cell
                ps_t = pps.tile([P, 512], F32, tag="pp")
                for e2 in range(EC // 2):
                    nc.tensor.matmul(
                        ps_t[:],
                        w_sb[:, 2 * e2 : 2 * e2 + 2, ts(pr, P)],
                        slab[:, 2 * e2 : 2 * e2 + 2, :],
                        start=(e2 == 0),
                        stop=(e2 == EC // 2 - 1),
                        perf_mode=mybir.MatmulPerfMode.DoubleRow,
                    )
                nc.vector.tensor_scalar_add(
                    st[:, pr, ts(tb, 512)], ps_t[:], b_t[:, pr : pr + 1]
                )

            def vproj_chunk(c, slab):
                ps_t = pps.tile([P, 512], F32, tag="pp")
                for e in range(EC):
                    nc.tensor.matmul(
                        ps_t[:],
                        slab[:, e, ts(c % 4, P)],
                        wv_sb[:, e, :],
                        start=(e == 0),
                        stop=False,
                    )
                nc.tensor.matmul(
                    ps_t[:], ones[:, :P], bv_row[:], start=False, stop=True
                )
                nc.vector.tensor_copy(
                    vp[:, c, :, 0:D],
                    ps_t.rearrange("p (h d) -> p h d", d=D),
                )

            def oproj_piece(tb, ob):
                ps_f = pps.tile([P, 512], F32, tag="pp")
                for dc in range(DC):
                    nc.tensor.matmul(
                        ps_f[:],
                        ao[:, dc, ts(tb, P)],
                        wo_sb[:, dc, ts(ob, 512)],
                        start=(dc == 0),
                        stop=(dc == DC - 1),
                    )
                o_t = ot.tile([P, 512], F32, tag="fout")
                nc.vector.tensor_copy(o_t[:], ps_f[:])
                nc.sync.dma_start(out[ts(tb, P), ts(ob, 512)], o_t[:])

            # normalization part 1 (block end): drain attnv PSUM into SBUF
            def norm_gather(ps_oo):
                sbp = npool.tile([P, 512], F32, tag="sbp", name="sbp")
                dn = npool.tile([1, 2, 512], F32, tag="dn", name="dn")
                nc.vector.tensor_copy(sbp[0:D, :], ps_oo[0][0:D, :])
                nc.vector.tensor_copy(sbp[D:P, :], ps_oo[1][0:D, :])
                nc.vector.tensor_copy(dn[:, 0, :], ps_oo[0][D : D + 1, :])
                nc.vector.tensor_copy(dn[:, 1, :], ps_oo[1][D : D + 1, :])
                return sbp, dn

            # normalization part 2 (mid next block): reciprocal + replicate
            def norm_finish(pr, qb, sbp, dn):
                dninv = npool.tile([1, 2, 512], F32, tag="dninv", name="dninv")
                nc.vector.reciprocal_approx_fast(dninv[:], dn[:])
                dinv = npool.tile([1, 2, 512], F16, tag="dinv", name="dinv")
                nc.vector.tensor_copy(dinv[:], dninv[:])
                ps_r = pps.tile([P, 512], F32, tag="pp", name="ps_r")
                nc.tensor.matmul(
                    ps_r[0:D, :], ones[:, :D], dinv[:, 0, :],
                    start=True, stop=True,
                )
                nc.tensor.matmul(
                    ps_r[D:P, :], ones[:, :D], dinv[:, 1, :],
                    start=True, stop=True, tile_position=(0, 64),
                )
                nc.vector.tensor_tensor(
                    ao[:, pr, ts(qb, 512)],
                    sbp[:],
                    ps_r[:],
                    mybir.AluOpType.mult,
                )

            # ---------------- startup ----------------
            # DMA issue order = criticality order: the minimal set gating
            # the first scores (wk + k-slab 0 + wq + q-slab 0 + biases)
            # goes first, then tensors needed progressively later.
            # Everything else (kproj tb1-3, remaining qproj, the whole v
            # projection) runs inside block (0,0) as PE filler while the
            # exp stream ramps.
            nc.sync.dma_start(bk_t[:], bqk[1])
            nc.sync.dma_start(wk_sb[:], wk)
            kslabs = [load_slab(xk, 0)]
            nc.sync.dma_start(bq_t[:], bqk[0])
            nc.sync.dma_start(wq_sb[:], wq)
            qslab0 = load_slab(xq, 0)
            nc.sync.dma_start(bv_row[:], bvr)
            nc.sync.dma_start(wv_sb[:], wv)
            vslabs = {0: load_vslab(0)}
            kslabs += [load_slab(xk, tb) for tb in range(1, TB)]
            vslabs[1] = load_vslab(1)
            qslabs_rest = {tb: load_slab(xq, tb) for tb in range(1, TB)}
            vslabs[2] = load_vslab(2)
            vslabs[3] = load_vslab(3)
            nc.sync.dma_start(wo_sb[:], wo)
            qk_chunk(0, ks, wk_sb, bk_t, 0, kslabs[0])
            qk_chunk(0, qs, wq_sb, bq_t, 0, qslab0)

            # block (0,0) filler sequence: v-projection chunks, remaining
            # k-projection (tb t needed by scores kt>=4t) and q-projection
            # (needed by blocks (0,1..3)); emitted 2 per kt
            blk00_tasks = []
            for c in range(16):
                blk00_tasks.append(
                    lambda c=c: vproj_chunk(c, vslabs[c // 4])
                )
                if c in (1, 3, 5):
                    tb = (c + 1) // 2
                    blk00_tasks.append(
                        lambda tb=tb: qk_chunk(
                            0, ks, wk_sb, bk_t, tb, kslabs[tb]
                        )
                    )
                if c in (7, 9, 11):
                    tb = (c - 5) // 2
                    blk00_tasks.append(
                        lambda tb=tb: qk_chunk(
                            0, qs, wq_sb, bq_t, tb, qslabs_rest[tb]
                        )
                    )

            # ---------------- attention with interleaved fillers ----------
            pending_norm = None     # (pr, qb, sbp, dn)

            for pr in range(DC):
                # filler tasks for this pair's 4 blocks: list of callables
                fillers = [[] for _ in range(TB)]
                if pr < DC - 1:
                    # next pair's q/k projection: 8 chunks
                    npr = pr + 1
                    chunks = []
                    for x_ap, w_sb, b_t, st in [
                        (xk, wk_sb, bk_t, ks),
                        (xq, wq_sb, bq_t, qs),
                    ]:
                        for tb in range(TB):
                            chunks.append((x_ap, w_sb, b_t, st, tb))

                    def mk_qk(npr, x_ap, w_sb, b_t, st, tb):
                        holder = {}

                        def prefetch():
                            holder["slab"] = load_slab(x_ap, tb)

                        def run():
                            qk_chunk(npr, st, w_sb, b_t, tb, holder["slab"])

                        return prefetch, run

                    start_qb = 0 if pr > 0 else 1
                    tasks = [(mk_qk(npr, *ch)) for ch in chunks]
                    nblk = TB - start_qb
                    for i, (pf, run) in enumerate(tasks):
                        blk = start_qb + min(i * nblk // len(tasks), nblk - 1)
                        fillers[blk].append((pf, run))
                else:
                    # pair 3: output projection for finished q-blocks
                    def mk_op(tb, ob):
                        return (None, lambda: oproj_piece(tb, ob))

                    for j in range(TB - 1):      # oproj for qb j in block j+1
                        for tb in range(4 * j, 4 * j + 4):
                            for ob in range(2):
                                fillers[j + 1].append(mk_op(tb, ob))

                for qb in range(TB):
                    # emit prefetches for this block's fillers up front
                    for pf, _ in fillers[qb]:
                        if pf is not None:
                            pf()
                    fq = [run for _, run in fillers[qb]]
                    fi = 0

                    ps_oo = [
                        ops.tile([P, 512], F32, tag=f"ov{i}", name=f"ov{i}")
                        for i in range(2)
                    ]
                    ets = {}
                    first_blk = pr == 0 and qb == 0
                    lag = 4 if first_blk else LAG
                    for kt in range(KT):
                        # scores for both heads of the pair (row-tiled pair)
                        ps_s = sps.tile([P, 1024], F32, tag="sc")
                        for i in range(2):
                            nc.tensor.matmul(
                                ps_s[:, ts(i, 512)],
                                ks[ds(64 * i, 64), pr, ts(kt, P)],
                                qs[ds(64 * i, 64), pr, ts(qb, 512)],
                                start=True,
                                stop=True,
                                tile_position=(64 * i, 0),
                            )
                        e_t = ep.tile([P, 1024], F16, tag="exp", name="e_t")
                        nc.scalar.activation(
                            e_t[:],
                            ps_s[:],
                            mybir.ActivationFunctionType.Exp,
                            scale=float(1.0 / (32.0 * WS * WS)),
                        )
                        ets[kt] = e_t

                        if first_blk:
                            # v projection + remaining k/q projection race
                            # ahead of the attnv consumption (lag 4)
                            for _ in range(2):
                                if blk00_tasks:
                                    blk00_tasks.pop(0)()
                        # pair 3's oproj fillers read ao written by
                        # norm_finish, so it must be emitted before them
                        nf_kt = 1 if pr == DC - 1 else 5
                        if kt == nf_kt and pending_norm is not None:
                            norm_finish(*pending_norm)
                            pending_norm = None
                        if (
                            kt % 2 == 1
                            and (pr != DC - 1 or kt >= 3)
                            and fi < len(fq)
                        ):
                            fq[fi]()
                            fi += 1
                        if kt >= lag:
                            k2 = kt - lag
                            for i in range(2):
                                nc.tensor.matmul(
                                    ps_oo[i][0 : D + 1, :],
                                    vp[:, k2, 2 * pr + i, :],
                                    ets[k2][:, ts(i, 512)],
                                    start=(k2 == 0),
                                    stop=False,
                                )
                    if first_blk:
                        while blk00_tasks:
                            blk00_tasks.pop(0)()
                    for k2 in range(KT - lag, KT):
                        for i in range(2):
                            nc.tensor.matmul(
                                ps_oo[i][0 : D + 1, :],
                                vp[:, k2, 2 * pr + i, :],
                                ets[k2][:, ts(i, 512)],
                                start=False,
                                stop=(k2 == KT - 1),
                            )
                    while fi < len(fq):
                        fq[fi]()
                        fi += 1
                    sbp, dn = norm_gather(ps_oo)
                    pending_norm = (pr, qb, sbp, dn)

            # tail: last block's normalization + its output projection
            norm_finish(*pending_norm)
            pending_norm = None
            for tb in range(4 * (TB - 1), 4 * TB):
                for ob in range(2):
                    oproj_piece(tb, ob)

    nc.compile()
    return nc


def _swizzle_x(xt, dtype):
    # [E, L] -> [P, TB, EC, 512]  with E = eo*P + p, L = tb*512 + t
    arr = xt.reshape(EC, P, TB, 512).transpose(1, 2, 0, 3)
    return np.ascontiguousarray(arr.astype(dtype))


def _swizzle_w(wt, dtype, inner):
    # [E_in, F] -> [P, E_in//P, F]
    arr = wt.reshape(inner, P, wt.shape[1]).transpose(1, 0, 2)
    return np.ascontiguousarray(arr.astype(dtype))


def kernel(q, k, v, padding_mask, sequence_mask, Wq, bq, Wk, bk, Wv, bv, Wo, bo):
    # masks intentionally unused: the reference discards masked_fill results.
    import ml_dtypes

    F8NP = ml_dtypes.float8_e4m3

    if "nc" not in _CACHE:
        _CACHE["nc"] = _build()
    nc = _CACHE["nc"]

    q = np.asarray(q, np.float32)
    k = np.asarray(k, np.float32)
    v = np.asarray(v, np.float32)
    Wq = np.asarray(Wq, np.float32)
    Wk = np.asarray(Wk, np.float32)
    Wv = np.asarray(Wv, np.float32)
    Wo = np.asarray(Wo, np.float32)
    bq = np.asarray(bq, np.float32)
    bk = np.asarray(bk, np.float32)
    bv = np.asarray(bv, np.float32)
    bo = np.asarray(bo, np.float32)

    in_maps = []
    for c in range(8):
        n, g = c // 2, c % 2
        sl = slice(g * GE, (g + 1) * GE)
        # q/k projections run in fp8 with weights pre-scaled by WS; the
        # scale cancels inside the softmax (folded into the exp scale).
        bqk_arr = np.stack(
            [
                (WS * bq[sl]).reshape(DC, P).T,
                (WS * bk[sl]).reshape(DC, P).T,
            ]
        ).astype(np.float32)
        in_maps.append(
            {
                "xq": _swizzle_x(q[n].T, F8NP),
                "xk": _swizzle_x(k[n].T, F8NP),
                "xv": _swizzle_x(v[n].T, np.float16),
                "wq": _swizzle_w(WS * Wq[sl, :].T, F8NP, EC),
                "wk": _swizzle_w(WS * Wk[sl, :].T, F8NP, EC),
                "wv": _swizzle_w(Wv[sl, :].T, np.float16, EC),
                "wo": _swizzle_w(Wo[:, sl].T, np.float16, DC),
                "bqk": np.ascontiguousarray(bqk_arr),
                "bvr": np.ascontiguousarray(bv[sl][None, :].astype(np.float16)),
            }
        )

    trace = os.environ.get("KERNEL_TRACE") == "1"
    kw = {}
    if trace:
        kw = dict(trace=True, trace_cores=list(range(8)))
    res = run_bass_kernel_spmd(nc, in_maps, core_ids=list(range(8)), **kw)
    if trace:
        _CACHE["exec_time_ns"] = res.exec_time_ns
        _CACHE["mean_exec_time_ns"] = res.mean_exec_time_ns

    outp = np.empty((NB, L, E), np.float32)
    for n in range(NB):
        outp[n] = (
            res.results[2 * n]["out"] + res.results[2 * n + 1]["out"] + bo[None, :]
        )
    return outp
